# revision 1
# baseline (speedup 1.0000x reference)
"""Trainium2 Bass kernel for nn_CrossAttentionGenerator.

Pipeline (per core; 8 cores = 4 batches x 2 halves of N):
  - MLP features for the core's source half (4096 rows) and the full target
    (8192 rows) of its batch: Linear(3->64) -> LayerNorm -> ReLU -> Linear(64->64).
    Target features + coords are written to a DRAM table for gathering.
  - Distance phase per 128-row source tile: v = 2 s.t - |t|^2 - |s|^2 (~ -dist)
    computed as an exact multi-bf16-split matmul (K=30, 1 cyc/row) into PSUM,
    evacuated to SBUF.
  - Top-8 selection per row (exact): nc.vector.max (top-8 values) +
    nc.vector.max_index (their positions).
  - Attention: gather [feat|coords] rows from the DRAM table via per-partition
    indirect DMA, logits = srcF . K / temp, softmax, attended = sum attn * V.

Self-contained: hardcodes B=4, N=M=8192, F=64, K=8 and does all sharding
host-side inside kernel().
"""

import numpy as np

import concourse.bacc as bacc
import concourse.bass as bass
import concourse.tile as tile
import concourse.mybir as mybir
from concourse.bass_utils import run_bass_kernel_spmd
from concourse.masks import make_identity

import ml_dtypes

bf16 = ml_dtypes.bfloat16
f32 = mybir.dt.float32
bf16_t = mybir.dt.bfloat16
u16 = mybir.dt.uint16
i16 = mybir.dt.int16
u32 = mybir.dt.uint32

B, N, M, F = 4, 8192, 8192, 64
K_NN = 8
N_SH = N // 2            # rows per core
TILES = N_SH // 128      # 32 source tiles
MTILES = M // 128        # 64 target tiles
K_SPLIT = 30             # split-bf16 distance matmul contraction size
TBL_W = 68               # feat(64) + coords(3) + pad(1)
LN_EPS = 1e-5

_CACHE = {}


def _split3(x):
    """fp32 -> three bf16 planes (h+m+l reconstructs x to ~2^-24 rel)."""
    h = x.astype(bf16).astype(np.float32)
    r = (x - h).astype(np.float32)
    m = r.astype(bf16).astype(np.float32)
    l = (r - m).astype(np.float32).astype(bf16).astype(np.float32)
    return h, m, l


def _build_dist_strips(s, t):
    """lhsT (30, n) and rhs (30, m) bf16 strips for v = 2 s.t - |t|^2 - |s|^2.

    Row order groups terms per coordinate so PSUM partial sums stay small
    (cancellation-friendly), matching the numpy-validated emulation:
      per c: (2sh,th) (2sh,tm) (2sm,th) (2sh,tl) (2sl,th) (2sm,tm) (2sm,tl) (2sl,tm)
      then (-1, t2h/m/l), (-s2h/m/l, 1).
    """
    n = s.shape[0]; m = t.shape[0]
    sh, sm, sl = _split3(2.0 * s)
    th, tm_, tl = _split3(t)
    tsq = ((t[:, 0] * t[:, 0]).astype(np.float32)
           + (t[:, 1] * t[:, 1]).astype(np.float32))
    tsq = (tsq.astype(np.float32) + (t[:, 2] * t[:, 2]).astype(np.float32)).astype(np.float32)
    ssq = ((s[:, 0] * s[:, 0]).astype(np.float32)
           + (s[:, 1] * s[:, 1]).astype(np.float32))
    ssq = (ssq.astype(np.float32) + (s[:, 2] * s[:, 2]).astype(np.float32)).astype(np.float32)
    t2h, t2m, t2l = _split3(tsq)
    s2h, s2m, s2l = _split3(ssq)
    ones_n = np.ones((n,), np.float32)
    ones_m = np.ones((m,), np.float32)

    lhs_rows = []
    rhs_rows = []
    for c in range(3):
        pairs = [(sh[:, c], th[:, c]), (sh[:, c], tm_[:, c]), (sm[:, c], th[:, c]),
                 (sh[:, c], tl[:, c]), (sl[:, c], th[:, c]), (sm[:, c], tm_[:, c]),
                 (sm[:, c], tl[:, c]), (sl[:, c], tm_[:, c])]
        for a, b in pairs:
            lhs_rows.append(a)
            rhs_rows.append(b)
    for r in (t2h, t2m, t2l):
        lhs_rows.append(-ones_n)
        rhs_rows.append(r)
    for r in (s2h, s2m, s2l):
        lhs_rows.append(-r)
        rhs_rows.append(ones_m)
    lhsT = np.stack(lhs_rows).astype(bf16)   # (30, n)
    rhs = np.stack(rhs_rows).astype(bf16)    # (30, m)
    return lhsT, rhs


def _build_program(trivial_ln):
    nc = bacc.Bacc("TRN2", target_bir_lowering=False, num_devices=8)

    # ---- I/O -------------------------------------------------------------
    lhsT_d = nc.dram_tensor("lhsT", [K_SPLIT, N_SH], bf16_t, kind="ExternalInput")
    rhs_d = nc.dram_tensor("rhs", [K_SPLIT, M], bf16_t, kind="ExternalInput")
    srcT4_d = nc.dram_tensor("srcT4", [4, N_SH], f32, kind="ExternalInput")
    tgtT4_d = nc.dram_tensor("tgtT4", [4, M], f32, kind="ExternalInput")
    w1b_d = nc.dram_tensor("w1b", [4, F], f32, kind="ExternalInput")
    w2b_d = nc.dram_tensor("w2b", [F + 1, F], f32, kind="ExternalInput")
    ltc_d = nc.dram_tensor("ltc", [128, 1], f32, kind="ExternalInput")
    if not trivial_ln:
        lng_d = nc.dram_tensor("lng", [128, F], f32, kind="ExternalInput")
        lnb_d = nc.dram_tensor("lnb", [128, F], f32, kind="ExternalInput")
    out_d = nc.dram_tensor("out", [N_SH, 3], f32, kind="ExternalOutput")

    with tile.TileContext(nc) as tc:
        import contextlib
        ctx = contextlib.ExitStack()
        with ctx:
            const = ctx.enter_context(tc.tile_pool(name="const", bufs=1))
            mlp_sb = ctx.enter_context(tc.tile_pool(name="mlp_sb", bufs=3))
            mlp_ps = ctx.enter_context(tc.tile_pool(name="mlp_ps", bufs=2, space="PSUM"))
            dist_ps = ctx.enter_context(tc.tile_pool(name="dist_ps", bufs=2, space="PSUM"))
            row_sb = ctx.enter_context(tc.tile_pool(name="row_sb", bufs=2))
            sel_sb = ctx.enter_context(tc.tile_pool(name="sel_sb", bufs=2))
            att_sb = ctx.enter_context(tc.tile_pool(name="att_sb", bufs=2))
            dram = ctx.enter_context(tc.tile_pool(name="dram", bufs=1, space="DRAM"))

            # ---- constants / strips -------------------------------------
            lhsT = const.tile([K_SPLIT, N_SH], bf16_t)
            nc.sync.dma_start(lhsT[:], lhsT_d[:])
            rhs = const.tile([K_SPLIT, M], bf16_t)
            nc.sync.dma_start(rhs[:], rhs_d[:])
            srcT4 = const.tile([4, N_SH], f32)
            nc.sync.dma_start(srcT4[:], srcT4_d[:])
            tgtT4 = const.tile([4, M], f32)
            nc.sync.dma_start(tgtT4[:], tgtT4_d[:])
            w1b = const.tile([4, F], f32)
            nc.sync.dma_start(w1b[:], w1b_d[:])
            w2b = const.tile([F + 1, F], f32)
            nc.sync.dma_start(w2b[:], w2b_d[:])
            ltc = const.tile([128, 1], f32)
            nc.sync.dma_start(ltc[:], ltc_d[:])
            if not trivial_ln:
                lng = const.tile([128, F], f32)
                nc.sync.dma_start(lng[:], lng_d[:])
                lnb = const.tile([128, F], f32)
                nc.sync.dma_start(lnb[:], lnb_d[:])

            ident = const.tile([128, 128], f32)
            make_identity(nc, ident[:])

            # inv temperature column: 0.125 * exp(-log_temp)
            invt = const.tile([128, 1], f32)
            nc.scalar.activation(invt[:], ltc[:], mybir.ActivationFunctionType.Exp,
                                 scale=-1.0)
            nc.vector.tensor_scalar_mul(invt[:], invt[:], 1.0 / 8.0)

            srcF = const.tile([128, TILES * F], f32)       # source features
            outacc = const.tile([128, TILES * 3], f32)     # attended accumulator

            # feature table in DRAM: [feat(64) | coords(3) | pad]
            ftable = dram.tile([M, TBL_W], f32)
            # coords columns from tgtT4 rows 0:3 (one strided DMA)
            nc.sync.dma_start(
                ftable[:, 64:67].rearrange("m c -> c m"), tgtT4[0:3, :])

            # ---- MLP over target tiles then source tiles ----------------
            def mlp_tile(xT4_ap, dst_kind, idx):
                h_ps = mlp_ps.tile([128, F], f32, tag="mm")
                nc.tensor.matmul(h_ps[:], lhsT=xT4_ap, rhs=w1b[:],
                                 start=True, stop=True)
                stats = mlp_sb.tile([128, 6], f32, tag="stats")
                nc.vector.bn_stats(out=stats[:], in_=h_ps[:])
                aggr = mlp_sb.tile([128, 2], f32, tag="aggr")
                nc.vector.bn_aggr(out=aggr[:], in_=stats[:])
                # istd = sqrt(1/(var+eps))
                vre = mlp_sb.tile([128, 1], f32, tag="vre")
                nc.vector.tensor_scalar_add(vre[:], aggr[:, 1:2], LN_EPS)
                nc.vector.reciprocal(vre[:], vre[:])
                istd = mlp_sb.tile([128, 1], f32, tag="istd")
                nc.scalar.activation(istd[:], vre[:], mybir.ActivationFunctionType.Sqrt)
                nmu = mlp_sb.tile([128, 1], f32, tag="nmu")
                nc.vector.scalar_tensor_tensor(
                    out=nmu[:], in0=aggr[:, 0:1], scalar=-1.0, in1=istd[:],
                    op0=mybir.AluOpType.mult, op1=mybir.AluOpType.mult)
                z = mlp_sb.tile([128, F], f32, tag="z")
                if trivial_ln:
                    nc.scalar.activation(z[:], h_ps[:],
                                         mybir.ActivationFunctionType.Relu,
                                         bias=nmu[:], scale=istd[:])
                else:
                    zn = mlp_sb.tile([128, F], f32, tag="zn")
                    nc.scalar.activation(zn[:], h_ps[:],
                                         mybir.ActivationFunctionType.Identity,
                                         bias=nmu[:], scale=istd[:])
                    nc.vector.scalar_tensor_tensor(
                        out=zn[:], in0=zn[:], scalar=1.0, in1=lng[:],
                        op0=mybir.AluOpType.mult, op1=mybir.AluOpType.mult)
                    nc.vector.scalar_tensor_tensor(
                        out=zn[:], in0=zn[:], scalar=0.0, in1=lnb[:],
                        op0=mybir.AluOpType.add, op1=mybir.AluOpType.add)
                    nc.vector.tensor_relu(z[:], zn[:])
                zt_ps = mlp_ps.tile([F, 128], f32, tag="tr")
                nc.tensor.transpose(out=zt_ps[:], in_=z[:], identity=ident[:])
                hT = mlp_sb.tile([F + 1, 128], f32, tag="hT")
                nc.scalar.copy(hT[0:F, :], zt_ps[:])
                nc.gpsimd.memset(hT[F:F + 1, :], 1.0)
                f_ps = mlp_ps.tile([128, F], f32, tag="mm")
                nc.tensor.matmul(f_ps[:], lhsT=hT[:], rhs=w2b[:],
                                 start=True, stop=True)
                if dst_kind == "tgt":
                    feat = mlp_sb.tile([128, F], f32, tag="feat")
                    nc.scalar.copy(feat[:], f_ps[:])
                    nc.sync.dma_start(ftable[idx * 128:(idx + 1) * 128, 0:F], feat[:])
                else:
                    nc.scalar.copy(srcF[:, idx * F:(idx + 1) * F], f_ps[:])

            for i in range(MTILES):
                mlp_tile(tgtT4[:, i * 128:(i + 1) * 128], "tgt", i)
            for i in range(TILES):
                mlp_tile(srcT4[:, i * 128:(i + 1) * 128], "src", i)

            # ---- distance + top-k + attention per source tile -----------
            NGRP = 8          # PSUM evacuation groups per tile
            GW = M // NGRP    # 1024 columns per group
            for t in range(TILES):
                lhs_t = lhsT[:, t * 128:(t + 1) * 128]
                row = row_sb.tile([128, M], f32, tag="row")
                for g in range(NGRP):
                    ps = dist_ps.tile([128, GW], f32, tag="d")
                    for h in range(GW // 512):
                        c0 = g * GW + h * 512
                        nc.tensor.matmul(ps[:, h * 512:(h + 1) * 512],
                                         lhsT=lhs_t,
                                         rhs=rhs[:, c0:c0 + 512],
                                         start=True, stop=True)
                    nc.scalar.copy(row[:, g * GW:(g + 1) * GW], ps[:])

                # exact top-8 (maximum v = closest)
                v8 = sel_sb.tile([128, 8], f32, tag="v8")
                nc.vector.max(out=v8[:], in_=row[:])
                m8 = sel_sb.tile([128, 8], u16, tag="m8")
                nc.vector.max_index(out=m8[:], in_max=v8[:], in_values=row[:])
                m8w = sel_sb.tile([128, 8], u32, tag="m8w")
                nc.vector.tensor_copy(out=m8w[:], in_=m8[:])

                # gather [feat|coords] for the 8 neighbours
                gath = att_sb.tile([128, K_NN, TBL_W], f32, tag="gath")
                for k in range(K_NN):
                    nc.gpsimd.indirect_dma_start(
                        out=gath[:, k, :], out_offset=None,
                        in_=ftable[:],
                        in_offset=bass.IndirectOffsetOnAxis(ap=m8w[:, k:k + 1], axis=0))

                # logits_k = srcF . K_k
                logits = att_sb.tile([128, K_NN], f32, tag="logits")
                scr = att_sb.tile([128, F], f32, tag="scr")
                sf = srcF[:, t * F:(t + 1) * F]
                for k in range(K_NN):
                    nc.vector.scalar_tensor_tensor(
                        out=scr[:], in0=gath[:, k, 0:F], scalar=1.0, in1=sf,
                        op0=mybir.AluOpType.mult, op1=mybir.AluOpType.mult,
                        accum_out=logits[:, k:k + 1])
                # softmax over k with temperature
                mx = att_sb.tile([128, 1], f32, tag="mx")
                nc.vector.tensor_reduce(out=mx[:], in_=logits[:],
                                        op=mybir.AluOpType.max,
                                        axis=mybir.AxisListType.X)
                bcol = att_sb.tile([128, 1], f32, tag="bcol")
                nc.vector.scalar_tensor_tensor(
                    out=bcol[:], in0=mx[:], scalar=-1.0, in1=invt[:],
                    op0=mybir.AluOpType.mult, op1=mybir.AluOpType.mult)
                att = att_sb.tile([128, K_NN], f32, tag="att")
                nc.scalar.activation(att[:], logits[:],
                                     mybir.ActivationFunctionType.Exp,
                                     bias=bcol[:], scale=invt[:])
                ssum = att_sb.tile([128, 1], f32, tag="ssum")
                nc.vector.tensor_reduce(out=ssum[:], in_=att[:],
                                        op=mybir.AluOpType.add,
                                        axis=mybir.AxisListType.X)
                rs = att_sb.tile([128, 1], f32, tag="rs")
                nc.vector.reciprocal(rs[:], ssum[:])
                nc.vector.tensor_scalar_mul(att[:], att[:], rs[:])
                # attended = sum_k att * V  (V = gathered coords)
                prod = att_sb.tile([128, K_NN, 3], f32, tag="prod")
                nc.vector.scalar_tensor_tensor(
                    out=prod[:], in0=gath[:, :, F:F + 3], scalar=1.0,
                    in1=att[:].to_broadcast([128, K_NN, 3]),
                    op0=mybir.AluOpType.mult, op1=mybir.AluOpType.mult)
                nc.vector.tensor_reduce(
                    out=outacc[:, t * 3:(t + 1) * 3],
                    in_=prod[:].rearrange("p k c -> p c k"),
                    op=mybir.AluOpType.add, axis=mybir.AxisListType.X)

            # ---- write output -------------------------------------------
            nc.sync.dma_start(
                out_d[:].rearrange("(t p) c -> p t c", p=128),
                outacc[:].rearrange("p (t c) -> p t c", c=3))

    nc.compile()
    return nc


def _get_program(trivial_ln):
    key = ("prog", trivial_ln)
    if key not in _CACHE:
        _CACHE[key] = _build_program(trivial_ln)
    return _CACHE[key]


def kernel(source, target, W1, b1, ln_g, ln_b, W2, b2, log_temp):
    source = np.ascontiguousarray(np.asarray(source, dtype=np.float32))
    target = np.ascontiguousarray(np.asarray(target, dtype=np.float32))
    W1 = np.asarray(W1, np.float32); b1 = np.asarray(b1, np.float32)
    ln_g = np.asarray(ln_g, np.float32); ln_b = np.asarray(ln_b, np.float32)
    W2 = np.asarray(W2, np.float32); b2 = np.asarray(b2, np.float32)
    log_temp = np.asarray(log_temp, np.float32)

    trivial_ln = bool(np.all(ln_g == 1.0) and np.all(ln_b == 0.0))
    nc = _get_program(trivial_ln)

    w1b = np.concatenate([W1, b1[None, :]], axis=0).astype(np.float32)       # (4, 64)
    w2b = np.concatenate([W2, b2[None, :]], axis=0).astype(np.float32)       # (65, 64)
    ltc = np.full((128, 1), float(log_temp[0]), np.float32)

    in_maps = []
    for c in range(8):
        b = c // 2
        h = c % 2
        s = source[b, h * N_SH:(h + 1) * N_SH]      # (4096, 3)
        t = target[b]                                # (8192, 3)
        lhsT, rhs = _build_dist_strips(s, t)
        srcT4 = np.concatenate([s.T, np.ones((1, N_SH), np.float32)], axis=0)
        tgtT4 = np.concatenate([t.T, np.ones((1, M), np.float32)], axis=0)
        im = {
            "lhsT": np.ascontiguousarray(lhsT),
            "rhs": np.ascontiguousarray(rhs),
            "srcT4": np.ascontiguousarray(srcT4.astype(np.float32)),
            "tgtT4": np.ascontiguousarray(tgtT4.astype(np.float32)),
            "w1b": w1b, "w2b": w2b, "ltc": ltc,
        }
        if not trivial_ln:
            im["lng"] = np.tile(ln_g[None, :], (128, 1)).astype(np.float32)
            im["lnb"] = np.tile(ln_b[None, :], (128, 1)).astype(np.float32)
        in_maps.append(im)

    global _last_in_maps
    _last_in_maps = in_maps
    res = run_bass_kernel_spmd(nc, in_maps, core_ids=list(range(8)))
    out = np.zeros((B, N, 3), np.float32)
    for c in range(8):
        b = c // 2
        h = c % 2
        out[b, h * N_SH:(h + 1) * N_SH] = res.results[c]["out"]
    return out

